# revision 1
# baseline (speedup 1.0000x reference)
"""Scatter-add of active-site feature rows into a dense (B, L, C) output,
distributed over 8 NeuronCores (data-parallel over the batch axis).

Core m owns flat output positions [m*8192, (m+1)*8192). Positions are
mapped to (group g, partition p, lane j) via  local = g*128*G + p*G + j
(p<128, j<G), so a group's output tile [128 partitions, G*512] stores to
DRAM with one contiguous G*2KB run per partition. On the host, rows are
bucketed by (core, g, j) "block" and padded to a uniform capacity Kc (the
runtime max block occupancy, rounded up to a multiple of 32 for DMA engine
fan-out); the lane count G is chosen per input to minimize Kc. On device
each block's [128, 512] output tile is a one-hot matmul

    out[p, c] = sum_k 1{lidx[k] == p} * feats[k, c]

which sums duplicate indices in fp32 PSUM and writes exact zeros for
untouched positions — every output element is produced by the kernel.
"""

import numpy as np

import concourse.bacc as bacc
import concourse.mybir as mybir
import concourse.tile as tile
from concourse.bass_utils import run_bass_kernel_spmd

N_CORES = 8
B = 16
L = 4096
C = 512
POS_PER_CORE = B * L // N_CORES  # 8192
import os
G_ENV = os.environ.get("K_G")  # force a specific G (testing only)
NBLK = 64  # blocks per core
# Buffer depths per G, sized to fit SBUF (ft/ot tiles are G*2KB/partition)
_BUFS = {2: (12, 8), 4: (10, 8), 8: (6, 4)}
FBUFS = int(os.environ.get("K_FBUFS", "0"))
OBUFS = int(os.environ.get("K_OBUFS", "0"))
CONST_RING = os.environ.get("K_CONST_RING", "sync")
MM_DTYPE = os.environ.get("K_MM_DTYPE", "float32")
COPY_ENG = os.environ.get("K_COPY", "dve")
STORE_MIX = int(os.environ.get("K_STORE_MIX", "0"))  # every Nth store on sync ring (0=off)

_PROGRAM_CACHE: dict = {}


def _build_program(CH: int, Kc: int, G: int, FBUFS: int, OBUFS: int):
    NGRP = 64 // G
    f32 = mybir.dt.float32
    nc = bacc.Bacc(
        "TRN2",
        target_bir_lowering=False,
        debug=False,
        enable_asserts=False,
        num_devices=N_CORES,
    )
    split = MM_DTYPE in ("bf16split", "fp16split")
    fdt = {"bf16split": mybir.dt.bfloat16, "fp16split": mybir.dt.float16}.get(MM_DTYPE, f32)
    fwidth = 2 * C if split else C  # hi+lo halves per block when split
    feats_d = [
        nc.dram_tensor(f"feats{ch}", [Kc, NBLK * fwidth], fdt, kind="ExternalInput")
        for ch in range(CH)
    ]
    lidx_d = [
        nc.dram_tensor(f"lidx{ch}", [Kc, NBLK], f32, kind="ExternalInput")
        for ch in range(CH)
    ]
    iota_d = nc.dram_tensor("iota", [128, 128], f32, kind="ExternalInput")
    out_d = nc.dram_tensor("out", [POS_PER_CORE, C], f32, kind="ExternalOutput")

    eq = mybir.AluOpType.is_equal

    with tile.TileContext(nc) as tc:
        with (
            tc.tile_pool(name="const", bufs=1) as constp,
            tc.tile_pool(name="fpool", bufs=FBUFS) as fpool,
            tc.tile_pool(name="opool", bufs=OBUFS) as opool,
            tc.tile_pool(name="mpool", bufs=6) as mpool,
            tc.tile_pool(name="psum", bufs=8, space="PSUM") as pspool,
        ):
            const_eng = nc.sync if CONST_RING == "sync" else nc.scalar
            iota_t = constp.tile([128, 128], f32)
            const_eng.dma_start(iota_t[:], iota_d.ap())
            lidx_t = constp.tile([Kc, CH * NBLK], f32)
            for ch in range(CH):
                const_eng.dma_start(
                    lidx_t[:, ch * NBLK : (ch + 1) * NBLK], lidx_d[ch].ap()
                )

            # out viewed as [g, p, j, c]: row = g*1024 + p*8 + j
            out_v = out_d.ap().rearrange("(g p j) c -> g p (j c)", p=128, j=G)
            for g in range(NGRP):
                ftiles = []
                for ch in range(CH):
                    ft = fpool.tile([Kc, G * fwidth], fdt, tag="ft")
                    nc.sync.dma_start(
                        ft[:], feats_d[ch].ap()[:, g * G * fwidth : (g + 1) * G * fwidth]
                    )
                    ftiles.append(ft)
                ot = opool.tile([128, G * C], f32)
                for j in range(G):
                    b = g * G + j
                    ps = pspool.tile([128, C], f32)
                    for ch in range(CH):
                        m = mpool.tile([Kc, 128], fdt)
                        nc.vector.tensor_scalar(
                            m[:],
                            iota_t[:Kc, :],
                            lidx_t[:, ch * NBLK + b : ch * NBLK + b + 1],
                            None,
                            op0=eq,
                        )
                        if split:
                            base = j * 2 * C
                            nc.tensor.matmul(
                                ps[:], m[:], ftiles[ch][:, base : base + C],
                                start=(ch == 0), stop=False,
                            )
                            nc.tensor.matmul(
                                ps[:], m[:], ftiles[ch][:, base + C : base + 2 * C],
                                start=False, stop=(ch == CH - 1),
                            )
                        else:
                            lhsT = m[:]
                            rhs = ftiles[ch][:, j * C : (j + 1) * C]
                            if MM_DTYPE == "float32r":
                                lhsT = lhsT.bitcast(mybir.dt.float32r)
                                rhs = rhs.bitcast(mybir.dt.float32r)
                            nc.tensor.matmul(
                                ps[:],
                                lhsT,
                                rhs,
                                start=(ch == 0),
                                stop=(ch == CH - 1),
                            )
                    if COPY_ENG == "mix" and j % 2 == 0:
                        nc.scalar.copy(ot[:, j * C : (j + 1) * C], ps[:])
                    else:
                        nc.vector.tensor_copy(ot[:, j * C : (j + 1) * C], ps[:])
                # store on the second HWDGE ring (ACT) to decouple from loads;
                # optionally rebalance a fraction onto the sync ring
                if STORE_MIX and g % STORE_MIX == STORE_MIX - 1:
                    nc.sync.dma_start(out_v[g], ot[:])
                else:
                    nc.scalar.dma_start(out_v[g], ot[:])

    nc.compile()
    return nc


def _block_decomposition(idx, G):
    core = idx >> 13  # // 8192
    local = idx & 8191
    g = local // (128 * G)  # position group
    rem = local % (128 * G)
    p = rem // G  # partition (position G-tuple)
    j = rem % G  # lane within tuple
    blk = g * G + j  # block id within core, 0..63
    gblk = core * NBLK + blk  # global block id, 0..511
    counts = np.bincount(gblk, minlength=N_CORES * NBLK)
    K = int(counts.max())
    CH = (K + 127) // 128
    Kc = -(-K // CH)  # ceil
    # Multiple of 32 keeps the HWDGE descriptor fan-out balanced across all
    # 16 SDMA engines (measured: Kc=92 concentrates loads on 4 engines and
    # costs +80 us; Kc=96 spreads them).
    Kc = (Kc + 31) & ~31
    return gblk, p, CH, Kc


def _prepare_inputs(input_features, site_indices):
    feats = np.ascontiguousarray(np.asarray(input_features, dtype=np.float32))
    idx = np.asarray(site_indices).astype(np.int64)
    n = idx.shape[0]
    assert feats.shape == (n, C)

    # The block composition (hence the padded capacity Kc) depends on the
    # lane count G; pick the G that minimizes transferred bytes for this
    # input, preferring larger DMA runs on ties.
    if G_ENV is not None:
        G = int(G_ENV)
        gblk, lpos, CH, Kc = _block_decomposition(idx, G)
    else:
        best = None
        for cand in (4, 2, 8):
            gblk_c, lpos_c, CH_c, Kc_c = _block_decomposition(idx, cand)
            if best is None or CH_c * Kc_c < best[0] * best[1]:
                best = (CH_c, Kc_c, cand, gblk_c, lpos_c)
        CH, Kc, G, gblk, lpos = best

    order = np.argsort(gblk, kind="stable")
    counts = np.bincount(gblk, minlength=N_CORES * NBLK)

    starts = np.zeros(N_CORES * NBLK, dtype=np.int64)
    np.cumsum(counts[:-1], out=starts[1:])
    slot = np.arange(n, dtype=np.int64) - np.repeat(starts, counts)

    g_sorted = gblk[order]
    core_s = g_sorted // NBLK
    blk_s = g_sorted % NBLK
    ch_s = slot // Kc
    k_s = slot - ch_s * Kc

    if MM_DTYPE in ("bf16split", "fp16split"):
        if MM_DTYPE == "bf16split":
            import ml_dtypes

            hdt = ml_dtypes.bfloat16
        else:
            hdt = np.float16
        feats_pack = np.zeros((N_CORES, CH, Kc, NBLK, 2, C), dtype=hdt)
        fs = feats[order]
        hi = fs.astype(hdt)
        lo = (fs - hi.astype(np.float32)).astype(hdt)
        feats_pack[core_s, ch_s, k_s, blk_s, 0, :] = hi
        feats_pack[core_s, ch_s, k_s, blk_s, 1, :] = lo
    else:
        feats_pack = np.zeros((N_CORES, CH, Kc, NBLK, C), dtype=np.float32)
        feats_pack[core_s, ch_s, k_s, blk_s, :] = feats[order]
    lidx_pack = np.full((N_CORES, CH, Kc, NBLK), -1.0, dtype=np.float32)
    lidx_pack[core_s, ch_s, k_s, blk_s] = lpos[order].astype(np.float32)

    iota = np.tile(np.arange(128, dtype=np.float32), (128, 1))

    in_maps = []
    for c in range(N_CORES):
        m = {"iota": iota}
        for ch in range(CH):
            m[f"feats{ch}"] = feats_pack[c, ch].reshape(Kc, -1)
            m[f"lidx{ch}"] = lidx_pack[c, ch]
        in_maps.append(m)
    return in_maps, CH, Kc, G


def run(input_features, site_indices, trace: bool = False):
    in_maps, CH, Kc, G = _prepare_inputs(input_features, site_indices)
    fbufs = FBUFS or _BUFS[G][0]
    obufs = OBUFS or _BUFS[G][1]
    key = (CH, Kc, G, fbufs, obufs, CONST_RING, MM_DTYPE, COPY_ENG, STORE_MIX)
    if key not in _PROGRAM_CACHE:
        _PROGRAM_CACHE[key] = _build_program(CH, Kc, G, fbufs, obufs)
    nc = _PROGRAM_CACHE[key]
    res = run_bass_kernel_spmd(nc, in_maps, list(range(N_CORES)), trace=trace)
    out = np.concatenate([res.results[c]["out"] for c in range(N_CORES)], axis=0)
    return out.reshape(B, L, C), res


def kernel(input_features, site_indices, batch_size, length):
    assert int(batch_size) == B and int(length) == L
    out, _ = run(input_features, site_indices, trace=False)
    return out



# revision 6
# speedup vs baseline: 1.2784x; 1.2784x over previous
"""Scatter-add of active-site feature rows into a dense (B, L, C) output,
distributed over 8 NeuronCores (data-parallel over the batch axis).

Core m owns flat output positions [m*8192, (m+1)*8192). Positions are
mapped to (group g, partition p, lane j) via  local = g*128*G + p*G + j
(p<128, j<G), so a group's output tile [128 partitions, G*512] stores to
DRAM with one contiguous G*2KB run per partition. On the host, rows are
bucketed by (core, g, j) "block" and padded to a uniform capacity Kc (the
runtime max block occupancy, rounded up to a multiple of 32 for DMA engine
fan-out); the lane count G is chosen per input to minimize Kc. On device
each block's [128, 512] output tile is a one-hot matmul

    out[p, c] = sum_k 1{lidx[k] == p} * feats[k, c]

which sums duplicate indices in fp32 PSUM and writes exact zeros for
untouched positions — every output element is produced by the kernel.
"""

import numpy as np

import concourse.bacc as bacc
import concourse.mybir as mybir
import concourse.tile as tile
from concourse.bass_utils import run_bass_kernel_spmd

N_CORES = 8
B = 16
L = 4096
C = 512
POS_PER_CORE = B * L // N_CORES  # 8192
import os
G_ENV = os.environ.get("K_G")  # force a specific G (testing only)
NBLK = 64  # blocks per core
# Buffer depths per G, sized to fit SBUF (ft/ot tiles are G*2KB/partition)
_BUFS = {2: (12, 8), 4: (10, 8), 8: (6, 4)}
FBUFS = int(os.environ.get("K_FBUFS", "0"))
OBUFS = int(os.environ.get("K_OBUFS", "0"))
CONST_RING = os.environ.get("K_CONST_RING", "sync")
MM_DTYPE = os.environ.get("K_MM_DTYPE", "fp16")
COPY_ENG = os.environ.get("K_COPY", "mix")
STORE_MIX = int(os.environ.get("K_STORE_MIX", "0"))  # every Nth store on sync ring (0=off)
LOAD_GRPS = int(os.environ.get("K_LOAD_GRPS", "2"))  # groups per feats dma_start

_PROGRAM_CACHE: dict = {}


def _build_program(CH: int, Kc: int, G: int, FBUFS: int, OBUFS: int):
    NGRP = 64 // G
    f32 = mybir.dt.float32
    nc = bacc.Bacc(
        "TRN2",
        target_bir_lowering=False,
        debug=False,
        enable_asserts=False,
        num_devices=N_CORES,
    )
    split = MM_DTYPE in ("bf16split", "fp16split")
    fdt = {
        "bf16split": mybir.dt.bfloat16,
        "fp16split": mybir.dt.float16,
        "fp16": mybir.dt.float16,
        "bf16": mybir.dt.bfloat16,
    }.get(MM_DTYPE, f32)
    fwidth = 2 * C if split else C  # hi+lo halves per block when split
    feats_d = [
        nc.dram_tensor(f"feats{ch}", [Kc, NBLK * fwidth], fdt, kind="ExternalInput")
        for ch in range(CH)
    ]
    lidx_d = [
        nc.dram_tensor(f"lidx{ch}", [Kc, NBLK], f32, kind="ExternalInput")
        for ch in range(CH)
    ]
    iota_d = nc.dram_tensor("iota", [128, 128], f32, kind="ExternalInput")
    out_d = nc.dram_tensor("out", [POS_PER_CORE, C], f32, kind="ExternalOutput")

    eq = mybir.AluOpType.is_equal

    with tile.TileContext(nc) as tc:
        with (
            tc.tile_pool(name="const", bufs=1) as constp,
            tc.tile_pool(name="fpool", bufs=FBUFS) as fpool,
            tc.tile_pool(name="opool", bufs=OBUFS) as opool,
            tc.tile_pool(name="mpool", bufs=6) as mpool,
            tc.tile_pool(name="psum", bufs=8, space="PSUM") as pspool,
        ):
            const_eng = nc.sync if CONST_RING == "sync" else nc.scalar
            iota_t = constp.tile([128, 128], f32)
            const_eng.dma_start(iota_t[:], iota_d.ap())
            lidx_t = constp.tile([Kc, CH * NBLK], f32)
            for ch in range(CH):
                const_eng.dma_start(
                    lidx_t[:, ch * NBLK : (ch + 1) * NBLK], lidx_d[ch].ap()
                )

            # out viewed as [g, p, j, c]: row = g*1024 + p*8 + j
            out_v = out_d.ap().rearrange("(g p j) c -> g p (j c)", p=128, j=G)
            for g0 in range(0, NGRP, LOAD_GRPS):
                ng = min(LOAD_GRPS, NGRP - g0)
                ftiles = []
                for ch in range(CH):
                    ft = fpool.tile([Kc, ng * G * fwidth], fdt, tag="ft")
                    nc.sync.dma_start(
                        ft[:],
                        feats_d[ch].ap()[:, g0 * G * fwidth : (g0 + ng) * G * fwidth],
                    )
                    ftiles.append(ft)
                for gi in range(ng):
                    g = g0 + gi
                    ot = opool.tile([128, G * C], f32)
                    for j in range(G):
                        b = g * G + j
                        ps = pspool.tile([128, C], f32)
                        for ch in range(CH):
                            m = mpool.tile([Kc, 128], fdt)
                            nc.vector.tensor_scalar(
                                m[:],
                                iota_t[:Kc, :],
                                lidx_t[:, ch * NBLK + b : ch * NBLK + b + 1],
                                None,
                                op0=eq,
                            )
                            if split:
                                base = (gi * G + j) * 2 * C
                                nc.tensor.matmul(
                                    ps[:], m[:], ftiles[ch][:, base : base + C],
                                    start=(ch == 0), stop=False,
                                )
                                nc.tensor.matmul(
                                    ps[:], m[:], ftiles[ch][:, base + C : base + 2 * C],
                                    start=False, stop=(ch == CH - 1),
                                )
                            else:
                                lhsT = m[:]
                                rhs = ftiles[ch][
                                    :, (gi * G + j) * C : (gi * G + j + 1) * C
                                ]
                                if MM_DTYPE == "float32r":
                                    lhsT = lhsT.bitcast(mybir.dt.float32r)
                                    rhs = rhs.bitcast(mybir.dt.float32r)
                                nc.tensor.matmul(
                                    ps[:],
                                    lhsT,
                                    rhs,
                                    start=(ch == 0),
                                    stop=(ch == CH - 1),
                                )
                        # PSUM -> SBUF evacuation split between ACT and DVE so
                        # neither copy stream becomes the critical path
                        if (COPY_ENG == "mix" and b % 2 == 0) or COPY_ENG == "act":
                            nc.scalar.copy(ot[:, j * C : (j + 1) * C], ps[:])
                        else:
                            nc.vector.tensor_copy(ot[:, j * C : (j + 1) * C], ps[:])
                    # store on the second HWDGE ring (ACT) to decouple from loads;
                    # optionally rebalance a fraction onto the sync ring
                    if STORE_MIX and g % STORE_MIX == STORE_MIX - 1:
                        nc.sync.dma_start(out_v[g], ot[:])
                    else:
                        nc.scalar.dma_start(out_v[g], ot[:])

    nc.compile()
    return nc


def _block_decomposition(idx, G):
    core = idx >> 13  # // 8192
    local = idx & 8191
    g = local // (128 * G)  # position group
    rem = local % (128 * G)
    p = rem // G  # partition (position G-tuple)
    j = rem % G  # lane within tuple
    blk = g * G + j  # block id within core, 0..63
    gblk = core * NBLK + blk  # global block id, 0..511
    counts = np.bincount(gblk, minlength=N_CORES * NBLK)
    K = int(counts.max())
    CH = (K + 127) // 128
    Kc = -(-K // CH)  # ceil
    # Multiple of 32 keeps the HWDGE descriptor fan-out balanced across all
    # 16 SDMA engines (measured: Kc=92 concentrates loads on 4 engines and
    # costs +80 us; Kc=96 spreads them).
    Kc = (Kc + 31) & ~31
    return gblk, p, CH, Kc


def _prepare_inputs(input_features, site_indices):
    feats = np.ascontiguousarray(np.asarray(input_features, dtype=np.float32))
    idx = np.asarray(site_indices).astype(np.int64)
    n = idx.shape[0]
    assert feats.shape == (n, C)

    # The block composition (hence the padded capacity Kc) depends on the
    # lane count G; pick the G that minimizes transferred bytes for this
    # input, preferring larger DMA runs on ties.
    if G_ENV is not None:
        G = int(G_ENV)
        gblk, lpos, CH, Kc = _block_decomposition(idx, G)
    else:
        best = None
        for cand in (4, 2, 8):
            gblk_c, lpos_c, CH_c, Kc_c = _block_decomposition(idx, cand)
            if best is None or CH_c * Kc_c < best[0] * best[1]:
                best = (CH_c, Kc_c, cand, gblk_c, lpos_c)
        CH, Kc, G, gblk, lpos = best

    order = np.argsort(gblk, kind="stable")
    counts = np.bincount(gblk, minlength=N_CORES * NBLK)

    starts = np.zeros(N_CORES * NBLK, dtype=np.int64)
    np.cumsum(counts[:-1], out=starts[1:])
    slot = np.arange(n, dtype=np.int64) - np.repeat(starts, counts)

    g_sorted = gblk[order]
    core_s = g_sorted // NBLK
    blk_s = g_sorted % NBLK
    ch_s = slot // Kc
    k_s = slot - ch_s * Kc

    if MM_DTYPE in ("bf16split", "fp16split"):
        if MM_DTYPE == "bf16split":
            import ml_dtypes

            hdt = ml_dtypes.bfloat16
        else:
            hdt = np.float16
        feats_pack = np.zeros((N_CORES, CH, Kc, NBLK, 2, C), dtype=hdt)
        fs = feats[order]
        hi = fs.astype(hdt)
        lo = (fs - hi.astype(np.float32)).astype(hdt)
        feats_pack[core_s, ch_s, k_s, blk_s, 0, :] = hi
        feats_pack[core_s, ch_s, k_s, blk_s, 1, :] = lo
    else:
        if MM_DTYPE == "fp16":
            pdt = np.float16
        elif MM_DTYPE == "bf16":
            import ml_dtypes

            pdt = ml_dtypes.bfloat16
        else:
            pdt = np.float32
        feats_pack = np.zeros((N_CORES, CH, Kc, NBLK, C), dtype=pdt)
        feats_pack[core_s, ch_s, k_s, blk_s, :] = feats[order].astype(pdt)
    lidx_pack = np.full((N_CORES, CH, Kc, NBLK), -1.0, dtype=np.float32)
    lidx_pack[core_s, ch_s, k_s, blk_s] = lpos[order].astype(np.float32)

    iota = np.tile(np.arange(128, dtype=np.float32), (128, 1))

    in_maps = []
    for c in range(N_CORES):
        m = {"iota": iota}
        for ch in range(CH):
            m[f"feats{ch}"] = feats_pack[c, ch].reshape(Kc, -1)
            m[f"lidx{ch}"] = lidx_pack[c, ch]
        in_maps.append(m)
    return in_maps, CH, Kc, G


def run(input_features, site_indices, trace: bool = False):
    in_maps, CH, Kc, G = _prepare_inputs(input_features, site_indices)
    if MM_DTYPE in ("fp16", "bf16"):
        # ft chunk = LOAD_GRPS*G KiB/partition (16-bit), ot = G*2 KiB
        fbufs = FBUFS or 6
        obufs = OBUFS or 8
    else:
        fbufs = FBUFS or _BUFS[G][0]
        obufs = OBUFS or _BUFS[G][1]
    key = (CH, Kc, G, fbufs, obufs, CONST_RING, MM_DTYPE, COPY_ENG, STORE_MIX, LOAD_GRPS)
    if key not in _PROGRAM_CACHE:
        _PROGRAM_CACHE[key] = _build_program(CH, Kc, G, fbufs, obufs)
    nc = _PROGRAM_CACHE[key]
    res = run_bass_kernel_spmd(nc, in_maps, list(range(N_CORES)), trace=trace)
    out = np.concatenate([res.results[c]["out"] for c in range(N_CORES)], axis=0)
    return out.reshape(B, L, C), res


def kernel(input_features, site_indices, batch_size, length):
    assert int(batch_size) == B and int(length) == L
    out, _ = run(input_features, site_indices, trace=False)
    return out



# revision 7
# speedup vs baseline: 1.2852x; 1.0054x over previous
"""Scatter-add of active-site feature rows into a dense (B, L, C) output,
distributed over 8 NeuronCores (data-parallel over the batch axis).

Core m owns flat output positions [m*8192, (m+1)*8192). Positions are
mapped to (group g, partition p, lane j) via  local = g*128*G + p*G + j
(p<128, j<G), so a group's output tile [128 partitions, G*512] stores to
DRAM with one contiguous G*2KB run per partition. On the host, rows are
bucketed by (core, g, j) "block" and padded to a uniform capacity Kc (the
runtime max block occupancy, rounded up to a multiple of 32 for DMA engine
fan-out); the lane count G is chosen per input to minimize Kc. On device
each block's [128, 512] output tile is a one-hot matmul

    out[p, c] = sum_k 1{lidx[k] == p} * feats[k, c]

which sums duplicate indices in fp32 PSUM and writes exact zeros for
untouched positions — every output element is produced by the kernel.
"""

import numpy as np

import concourse.bacc as bacc
import concourse.mybir as mybir
import concourse.tile as tile
from concourse.bass_utils import run_bass_kernel_spmd

N_CORES = 8
B = 16
L = 4096
C = 512
POS_PER_CORE = B * L // N_CORES  # 8192
import os
G_ENV = os.environ.get("K_G")  # force a specific G (testing only)
NBLK = 64  # blocks per core
# Buffer depths per G, sized to fit SBUF (ft/ot tiles are G*2KB/partition)
_BUFS = {2: (12, 8), 4: (10, 8), 8: (6, 4)}
FBUFS = int(os.environ.get("K_FBUFS", "0"))
OBUFS = int(os.environ.get("K_OBUFS", "0"))
CONST_RING = os.environ.get("K_CONST_RING", "sync")
MM_DTYPE = os.environ.get("K_MM_DTYPE", "fp16")
COPY_ENG = os.environ.get("K_COPY", "mix")
STORE_MIX = int(os.environ.get("K_STORE_MIX", "0"))  # every Nth store on sync ring (0=off)
LOAD_GRPS = int(os.environ.get("K_LOAD_GRPS", "2"))  # groups per feats dma_start

_PROGRAM_CACHE: dict = {}


def _build_program(CH: int, Kc: int, G: int, FBUFS: int, OBUFS: int):
    NGRP = 64 // G
    f32 = mybir.dt.float32
    nc = bacc.Bacc(
        "TRN2",
        target_bir_lowering=False,
        debug=False,
        enable_asserts=False,
        num_devices=N_CORES,
    )
    split = MM_DTYPE in ("bf16split", "fp16split")
    fdt = {
        "bf16split": mybir.dt.bfloat16,
        "fp16split": mybir.dt.float16,
        "fp16": mybir.dt.float16,
        "bf16": mybir.dt.bfloat16,
    }.get(MM_DTYPE, f32)
    fwidth = 2 * C if split else C  # hi+lo halves per block when split
    feats_d = [
        nc.dram_tensor(f"feats{ch}", [Kc, NBLK * fwidth], fdt, kind="ExternalInput")
        for ch in range(CH)
    ]
    lidx_d = [
        nc.dram_tensor(f"lidx{ch}", [Kc, NBLK], f32, kind="ExternalInput")
        for ch in range(CH)
    ]
    iota_d = nc.dram_tensor("iota", [128, 128], f32, kind="ExternalInput")
    out_d = nc.dram_tensor("out", [POS_PER_CORE, C], f32, kind="ExternalOutput")

    eq = mybir.AluOpType.is_equal

    with tile.TileContext(nc) as tc:
        with (
            tc.tile_pool(name="const", bufs=1) as constp,
            tc.tile_pool(name="fpool", bufs=FBUFS) as fpool,
            tc.tile_pool(name="opool", bufs=OBUFS) as opool,
            tc.tile_pool(name="mpool", bufs=6) as mpool,
            tc.tile_pool(name="psum", bufs=8, space="PSUM") as pspool,
        ):
            const_eng = nc.sync if CONST_RING == "sync" else nc.scalar
            iota_t = constp.tile([128, 128], f32)
            lidx_t = constp.tile([Kc, CH * NBLK], f32)

            # First chunk is a single group so the first store fires as early
            # as possible; later chunks are LOAD_GRPS groups each.
            chunks = []
            g0 = 0
            while g0 < NGRP:
                ng = 1 if (g0 == 0 and LOAD_GRPS > 1) else min(LOAD_GRPS, NGRP - g0)
                chunks.append((g0, ng))
                g0 += ng

            # out viewed as [g, p, j, c]: row = g*1024 + p*8 + j
            out_v = out_d.ap().rearrange("(g p j) c -> g p (j c)", p=128, j=G)
            for ci, (g0, ng) in enumerate(chunks):
                ftiles = []
                for ch in range(CH):
                    ft = fpool.tile([Kc, ng * G * fwidth], fdt, tag="ft")
                    nc.sync.dma_start(
                        ft[:],
                        feats_d[ch].ap()[:, g0 * G * fwidth : (g0 + ng) * G * fwidth],
                    )
                    ftiles.append(ft)
                if ci == 0:
                    # consts issued after the first feats chunk: they are tiny
                    # and arrive well before the first is_equal needs them
                    const_eng.dma_start(iota_t[:], iota_d.ap())
                    for ch in range(CH):
                        const_eng.dma_start(
                            lidx_t[:, ch * NBLK : (ch + 1) * NBLK], lidx_d[ch].ap()
                        )
                for gi in range(ng):
                    g = g0 + gi
                    ot = opool.tile([128, G * C], f32)
                    for j in range(G):
                        b = g * G + j
                        ps = pspool.tile([128, C], f32)
                        for ch in range(CH):
                            m = mpool.tile([Kc, 128], fdt)
                            nc.vector.tensor_scalar(
                                m[:],
                                iota_t[:Kc, :],
                                lidx_t[:, ch * NBLK + b : ch * NBLK + b + 1],
                                None,
                                op0=eq,
                            )
                            if split:
                                base = (gi * G + j) * 2 * C
                                nc.tensor.matmul(
                                    ps[:], m[:], ftiles[ch][:, base : base + C],
                                    start=(ch == 0), stop=False,
                                )
                                nc.tensor.matmul(
                                    ps[:], m[:], ftiles[ch][:, base + C : base + 2 * C],
                                    start=False, stop=(ch == CH - 1),
                                )
                            else:
                                lhsT = m[:]
                                rhs = ftiles[ch][
                                    :, (gi * G + j) * C : (gi * G + j + 1) * C
                                ]
                                if MM_DTYPE == "float32r":
                                    lhsT = lhsT.bitcast(mybir.dt.float32r)
                                    rhs = rhs.bitcast(mybir.dt.float32r)
                                nc.tensor.matmul(
                                    ps[:],
                                    lhsT,
                                    rhs,
                                    start=(ch == 0),
                                    stop=(ch == CH - 1),
                                )
                        # PSUM -> SBUF evacuation split between ACT and DVE so
                        # neither copy stream becomes the critical path
                        if (COPY_ENG == "mix" and b % 2 == 0) or COPY_ENG == "act":
                            nc.scalar.copy(ot[:, j * C : (j + 1) * C], ps[:])
                        else:
                            nc.vector.tensor_copy(ot[:, j * C : (j + 1) * C], ps[:])
                    # store on the second HWDGE ring (ACT) to decouple from loads;
                    # optionally rebalance a fraction onto the sync ring
                    if STORE_MIX and g % STORE_MIX == STORE_MIX - 1:
                        nc.sync.dma_start(out_v[g], ot[:])
                    else:
                        nc.scalar.dma_start(out_v[g], ot[:])

    nc.compile()
    return nc


def _block_decomposition(idx, G):
    core = idx >> 13  # // 8192
    local = idx & 8191
    g = local // (128 * G)  # position group
    rem = local % (128 * G)
    p = rem // G  # partition (position G-tuple)
    j = rem % G  # lane within tuple
    blk = g * G + j  # block id within core, 0..63
    gblk = core * NBLK + blk  # global block id, 0..511
    counts = np.bincount(gblk, minlength=N_CORES * NBLK)
    K = int(counts.max())
    CH = (K + 127) // 128
    Kc = -(-K // CH)  # ceil
    # Multiple of 32 keeps the HWDGE descriptor fan-out balanced across all
    # 16 SDMA engines (measured: Kc=92 concentrates loads on 4 engines and
    # costs +80 us; Kc=96 spreads them).
    Kc = (Kc + 31) & ~31
    return gblk, p, CH, Kc


def _prepare_inputs(input_features, site_indices):
    feats = np.ascontiguousarray(np.asarray(input_features, dtype=np.float32))
    idx = np.asarray(site_indices).astype(np.int64)
    n = idx.shape[0]
    assert feats.shape == (n, C)

    # The block composition (hence the padded capacity Kc) depends on the
    # lane count G; pick the G that minimizes transferred bytes for this
    # input, preferring larger DMA runs on ties.
    if G_ENV is not None:
        G = int(G_ENV)
        gblk, lpos, CH, Kc = _block_decomposition(idx, G)
    else:
        best = None
        for cand in (4, 2, 8):
            gblk_c, lpos_c, CH_c, Kc_c = _block_decomposition(idx, cand)
            if best is None or CH_c * Kc_c < best[0] * best[1]:
                best = (CH_c, Kc_c, cand, gblk_c, lpos_c)
        CH, Kc, G, gblk, lpos = best

    order = np.argsort(gblk, kind="stable")
    counts = np.bincount(gblk, minlength=N_CORES * NBLK)

    starts = np.zeros(N_CORES * NBLK, dtype=np.int64)
    np.cumsum(counts[:-1], out=starts[1:])
    slot = np.arange(n, dtype=np.int64) - np.repeat(starts, counts)

    g_sorted = gblk[order]
    core_s = g_sorted // NBLK
    blk_s = g_sorted % NBLK
    ch_s = slot // Kc
    k_s = slot - ch_s * Kc

    if MM_DTYPE in ("bf16split", "fp16split"):
        if MM_DTYPE == "bf16split":
            import ml_dtypes

            hdt = ml_dtypes.bfloat16
        else:
            hdt = np.float16
        feats_pack = np.zeros((N_CORES, CH, Kc, NBLK, 2, C), dtype=hdt)
        fs = feats[order]
        hi = fs.astype(hdt)
        lo = (fs - hi.astype(np.float32)).astype(hdt)
        feats_pack[core_s, ch_s, k_s, blk_s, 0, :] = hi
        feats_pack[core_s, ch_s, k_s, blk_s, 1, :] = lo
    else:
        if MM_DTYPE == "fp16":
            pdt = np.float16
        elif MM_DTYPE == "bf16":
            import ml_dtypes

            pdt = ml_dtypes.bfloat16
        else:
            pdt = np.float32
        feats_pack = np.zeros((N_CORES, CH, Kc, NBLK, C), dtype=pdt)
        feats_pack[core_s, ch_s, k_s, blk_s, :] = feats[order].astype(pdt)
    lidx_pack = np.full((N_CORES, CH, Kc, NBLK), -1.0, dtype=np.float32)
    lidx_pack[core_s, ch_s, k_s, blk_s] = lpos[order].astype(np.float32)

    iota = np.tile(np.arange(128, dtype=np.float32), (128, 1))

    in_maps = []
    for c in range(N_CORES):
        m = {"iota": iota}
        for ch in range(CH):
            m[f"feats{ch}"] = feats_pack[c, ch].reshape(Kc, -1)
            m[f"lidx{ch}"] = lidx_pack[c, ch]
        in_maps.append(m)
    return in_maps, CH, Kc, G


def run(input_features, site_indices, trace: bool = False):
    in_maps, CH, Kc, G = _prepare_inputs(input_features, site_indices)
    if MM_DTYPE in ("fp16", "bf16"):
        # ft chunk = LOAD_GRPS*G KiB/partition (16-bit), ot = G*2 KiB
        fbufs = FBUFS or 6
        obufs = OBUFS or 8
    else:
        fbufs = FBUFS or _BUFS[G][0]
        obufs = OBUFS or _BUFS[G][1]
    key = (CH, Kc, G, fbufs, obufs, CONST_RING, MM_DTYPE, COPY_ENG, STORE_MIX, LOAD_GRPS)
    if key not in _PROGRAM_CACHE:
        _PROGRAM_CACHE[key] = _build_program(CH, Kc, G, fbufs, obufs)
    nc = _PROGRAM_CACHE[key]
    res = run_bass_kernel_spmd(nc, in_maps, list(range(N_CORES)), trace=trace)
    out = np.concatenate([res.results[c]["out"] for c in range(N_CORES)], axis=0)
    return out.reshape(B, L, C), res


def kernel(input_features, site_indices, batch_size, length):
    assert int(batch_size) == B and int(length) == L
    out, _ = run(input_features, site_indices, trace=False)
    return out



# revision 28
# speedup vs baseline: 1.2967x; 1.0089x over previous
"""Scatter-add of active-site feature rows into a dense (B, L, C) output,
distributed over 8 NeuronCores (data-parallel over the batch axis).

Core m owns flat output positions [m*8192, (m+1)*8192). Positions are
mapped to (group g, partition p, lane j) via  local = g*128*G + p*G + j
(p<128, j<G), so a group's output tile [128 partitions, G*512] stores to
DRAM with one contiguous G*2KB run per partition. On the host, rows are
bucketed by (core, g, j) "block" and padded to a uniform capacity Kc (the
runtime max block occupancy, rounded up to a multiple of 32 for DMA engine
fan-out); the lane count G is chosen per input to minimize Kc. On device
each block's [128, 512] output tile is a one-hot matmul

    out[p, c] = sum_k 1{lidx[k] == p} * feats[k, c]

which sums duplicate indices in fp32 PSUM and writes exact zeros for
untouched positions — every output element is produced by the kernel.

Features travel as fp16 (loads halve to ~6.4 MB/core and the PE streams
4x faster than fp32; quantization error ~2e-4 against the 2e-2 gate while
the one-hot stays exact and accumulation is fp32 PSUM). PSUM->SBUF
evacuation alternates between ACT and DVE so neither engine's copy stream
paces the stores. Loads ride the SP HWDGE ring, stores the ACT ring;
steady-state the 16 SDMA engines sustain ~420 GB/s combined r+w, which is
the roofline for the 16.8 MB/core of mandatory fp32 output writes plus
the fp16 feature reads.
"""

import numpy as np

import concourse.bacc as bacc
import concourse.mybir as mybir
import concourse.tile as tile
from concourse.bass_utils import run_bass_kernel_spmd

N_CORES = 8
B = 16
L = 4096
C = 512
POS_PER_CORE = B * L // N_CORES  # 8192
import os
G_ENV = os.environ.get("K_G")  # force a specific G (testing only)
NBLK = 64  # blocks per core
# Buffer depths per G, sized to fit SBUF (ft/ot tiles are G*2KB/partition)
_BUFS = {2: (12, 8), 4: (10, 8), 8: (6, 4)}
FBUFS = int(os.environ.get("K_FBUFS", "0"))
OBUFS = int(os.environ.get("K_OBUFS", "0"))
CONST_RING = os.environ.get("K_CONST_RING", "sync")
MM_DTYPE = os.environ.get("K_MM_DTYPE", "fp16")
COPY_ENG = os.environ.get("K_COPY", "mix")
STORE_MIX = int(os.environ.get("K_STORE_MIX", "0"))  # every Nth store on sync ring (0=off)
LOAD_GRPS = int(os.environ.get("K_LOAD_GRPS", "1"))  # groups per feats dma_start
EQ_ENG = os.environ.get("K_EQ_ENG", "dve")  # engine for the one-hot is_equal
COPY_SCN = int(os.environ.get("K_COPY_SCN", "2"))  # 1-in-N copies go to ACT in mix mode
PBK = os.environ.get("K_PBK", "0") == "1"  # per-block capacities (ragged packing)
NLB = int(os.environ.get("K_NLB", "2"))  # blocks per load dma in PBK mode
ALT_LOADS = int(os.environ.get("K_ALT_LOADS", "0"))  # first N chunks alternate rings
EARLY_SPLIT = int(os.environ.get("K_EARLY_SPLIT", "0"))  # first N groups store in halves
CPAIR = os.environ.get("K_CPAIR", "0") == "1"  # one copy per two PSUM banks

_PROGRAM_CACHE: dict = {}


def _build_program_pbk(kg: tuple, kbe: tuple, G: int, FBUFS: int, OBUFS: int):
    """Per-block-capacity variant: feats packed raggedly in DRAM (block b's
    rows padded only to this input's per-load-group capacity kg[t]), one
    load dma per NLB-block group, one single-chunk matmul per block over
    kbe[b] <= 128 rows. Compiled per input (the offsets are baked in)."""
    NGRP = 64 // G
    f32 = mybir.dt.float32
    fdt = mybir.dt.bfloat16 if MM_DTYPE == "bf16" else mybir.dt.float16
    nc = bacc.Bacc(
        "TRN2",
        target_bir_lowering=False,
        debug=False,
        enable_asserts=False,
        num_devices=N_CORES,
    )
    total_rows = sum(NLB * k for k in kg)
    feats_d = nc.dram_tensor("feats", [total_rows, C], fdt, kind="ExternalInput")
    lidx_d = nc.dram_tensor("lidx", [128, NBLK], f32, kind="ExternalInput")
    iota_d = nc.dram_tensor("iota", [128, 128], f32, kind="ExternalInput")
    out_d = nc.dram_tensor("out", [POS_PER_CORE, C], f32, kind="ExternalOutput")
    offs = [0]
    for k in kg:
        offs.append(offs[-1] + NLB * k)

    eq = mybir.AluOpType.is_equal

    with tile.TileContext(nc) as tc:
        with (
            tc.tile_pool(name="const", bufs=1) as constp,
            tc.tile_pool(name="fpool", bufs=FBUFS) as fpool,
            tc.tile_pool(name="opool", bufs=OBUFS) as opool,
            tc.tile_pool(name="mpool", bufs=6) as mpool,
            tc.tile_pool(name="psum", bufs=8, space="PSUM") as pspool,
        ):
            const_eng = nc.sync if CONST_RING == "sync" else nc.scalar
            iota_t = constp.tile([128, 128], f32)
            lidx_t = constp.tile([128, NBLK], f32)

            # out viewed as [g, p, j, c]: row = g*128*G + p*G + j
            out_v = out_d.ap().rearrange("(g p j) c -> g p (j c)", p=128, j=G)
            ft = None
            for g in range(NGRP):
                ot = opool.tile([128, G * C], f32)
                for j in range(G):
                    b = g * G + j
                    t, n = divmod(b, NLB)
                    if n == 0:
                        Kg = kg[t]
                        ft = fpool.tile([Kg, NLB * C], fdt, tag="ft")
                        src = feats_d.ap()[
                            offs[t] : offs[t] + NLB * Kg, :
                        ].rearrange("(n k) c -> k n c", k=Kg)
                        dst = ft[:].rearrange("k (n c) -> k n c", c=C)
                        nc.sync.dma_start(dst, src)
                        if b == 0:
                            # consts issued after the first feats chunk
                            const_eng.dma_start(iota_t[:], iota_d.ap())
                            const_eng.dma_start(lidx_t[:], lidx_d.ap())
                    K = kbe[b]
                    m = mpool.tile([K, 128], fdt)
                    nc.vector.tensor_scalar(
                        m[:], iota_t[:K, :], lidx_t[:K, b : b + 1], None, op0=eq
                    )
                    ps = pspool.tile([128, C], f32)
                    nc.tensor.matmul(
                        ps[:], m[:], ft[:K, n * C : (n + 1) * C],
                        start=True, stop=True,
                    )
                    if (COPY_ENG == "mix" and b % COPY_SCN == 0) or COPY_ENG == "act":
                        nc.scalar.copy(ot[:, j * C : (j + 1) * C], ps[:])
                    else:
                        nc.vector.tensor_copy(ot[:, j * C : (j + 1) * C], ps[:])
                if STORE_MIX and g % STORE_MIX == STORE_MIX - 1:
                    nc.sync.dma_start(out_v[g], ot[:])
                else:
                    nc.scalar.dma_start(out_v[g], ot[:])

    nc.compile()
    return nc


def _prepare_inputs_pbk(feats, idx, G):
    """Ragged packing for the per-block-capacity program. Returns None if
    any block overflows one matmul chunk (fall back to the uniform path)."""
    n = idx.shape[0]
    core = idx >> 13
    local = idx & 8191
    rem = local % (128 * G)
    blk = (local // (128 * G)) * G + rem % G
    lpos = rem // G
    gblk = core * NBLK + blk
    counts = np.bincount(gblk, minlength=N_CORES * NBLK).reshape(N_CORES, NBLK)
    if counts.max() > 128:
        return None
    ngl = NBLK // NLB
    # load-group capacity: max over cores and member blocks, mult of 8
    kg = counts.reshape(N_CORES, ngl, NLB).max(axis=(0, 2))
    kg = np.maximum((kg + 7) & ~7, 8)
    # per-block matmul extent: max over cores only
    kbe = counts.max(axis=0)
    kbe = np.maximum((kbe + 7) & ~7, 8)
    offs = np.zeros(ngl, dtype=np.int64)
    np.cumsum(NLB * kg[:-1], out=offs[1:])
    total_rows = int(offs[-1] + NLB * kg[-1])

    order = np.argsort(gblk, kind="stable")
    flat_counts = counts.ravel()
    starts = np.zeros(N_CORES * NBLK, dtype=np.int64)
    np.cumsum(flat_counts[:-1], out=starts[1:])
    slot = np.arange(n, dtype=np.int64) - np.repeat(starts, flat_counts)

    g_sorted = gblk[order]
    core_s = g_sorted // NBLK
    blk_s = g_sorted % NBLK
    t_s, n_s = np.divmod(blk_s, NLB)
    row_s = offs[t_s] + n_s * kg[t_s] + slot

    pdt = np.float16
    if MM_DTYPE == "bf16":
        import ml_dtypes

        pdt = ml_dtypes.bfloat16
    feats_pack = np.zeros((N_CORES, total_rows, C), dtype=pdt)
    feats_pack[core_s, row_s, :] = feats[order].astype(pdt)
    lidx_pack = np.full((N_CORES, 128, NBLK), -1.0, dtype=np.float32)
    lidx_pack[core_s, slot, blk_s] = lpos[order].astype(np.float32)

    iota = np.tile(np.arange(128, dtype=np.float32), (128, 1))
    in_maps = [
        {"iota": iota, "feats": feats_pack[c], "lidx": lidx_pack[c]}
        for c in range(N_CORES)
    ]
    return in_maps, tuple(int(k) for k in kg), tuple(int(k) for k in kbe)


def _build_program(CH: int, Kc: int, G: int, FBUFS: int, OBUFS: int):
    NGRP = 64 // G
    f32 = mybir.dt.float32
    nc = bacc.Bacc(
        "TRN2",
        target_bir_lowering=False,
        debug=False,
        enable_asserts=False,
        num_devices=N_CORES,
    )
    split = MM_DTYPE in ("bf16split", "fp16split")
    fdt = {
        "bf16split": mybir.dt.bfloat16,
        "fp16split": mybir.dt.float16,
        "fp16": mybir.dt.float16,
        "bf16": mybir.dt.bfloat16,
    }.get(MM_DTYPE, f32)
    fwidth = 2 * C if split else C  # hi+lo halves per block when split
    feats_d = [
        nc.dram_tensor(f"feats{ch}", [Kc, NBLK * fwidth], fdt, kind="ExternalInput")
        for ch in range(CH)
    ]
    lidx_d = [
        nc.dram_tensor(f"lidx{ch}", [Kc, NBLK], f32, kind="ExternalInput")
        for ch in range(CH)
    ]
    iota_d = nc.dram_tensor("iota", [128, 128], f32, kind="ExternalInput")
    out_d = nc.dram_tensor("out", [POS_PER_CORE, C], f32, kind="ExternalOutput")

    eq = mybir.AluOpType.is_equal

    with tile.TileContext(nc) as tc:
        with (
            tc.tile_pool(name="const", bufs=1) as constp,
            tc.tile_pool(name="fpool", bufs=FBUFS) as fpool,
            tc.tile_pool(name="opool", bufs=OBUFS) as opool,
            tc.tile_pool(name="mpool", bufs=6) as mpool,
            tc.tile_pool(name="psum", bufs=4 if CPAIR else 8, space="PSUM") as pspool,
        ):
            const_eng = nc.sync if CONST_RING == "sync" else nc.scalar
            iota_t = constp.tile([128, 128], f32)
            lidx_t = constp.tile([Kc, CH * NBLK], f32)

            # First chunk is a single group so the first store fires as early
            # as possible; later chunks are LOAD_GRPS groups each.
            chunks = []
            g0 = 0
            while g0 < NGRP:
                ng = 1 if (g0 == 0 and LOAD_GRPS > 1) else min(LOAD_GRPS, NGRP - g0)
                chunks.append((g0, ng))
                g0 += ng

            # out viewed as [g, p, j, c]: row = g*1024 + p*8 + j
            out_v = out_d.ap().rearrange("(g p j) c -> g p (j c)", p=128, j=G)
            for ci, (g0, ng) in enumerate(chunks):
                ftiles = []
                # during the read-only ramp, alternate the load ring so both
                # HWDGE queues feed the SDMA engines (reads-alone run ~60% of
                # the mixed rate on a single queue)
                ring = nc.scalar if (ci < ALT_LOADS and ci % 2 == 1) else nc.sync
                for ch in range(CH):
                    ft = fpool.tile([Kc, ng * G * fwidth], fdt, tag="ft")
                    ring.dma_start(
                        ft[:],
                        feats_d[ch].ap()[:, g0 * G * fwidth : (g0 + ng) * G * fwidth],
                    )
                    ftiles.append(ft)
                if ci == 0:
                    # consts issued after the first feats chunk: they are tiny
                    # and arrive well before the first is_equal needs them
                    const_eng.dma_start(iota_t[:], iota_d.ap())
                    for ch in range(CH):
                        const_eng.dma_start(
                            lidx_t[:, ch * NBLK : (ch + 1) * NBLK], lidx_d[ch].ap()
                        )
                for gi in range(ng):
                    g = g0 + gi
                    ot = opool.tile([128, G * C], f32)
                    ps2 = None
                    for j in range(G):
                        b = g * G + j
                        if CPAIR and G >= 2:
                            if j % 2 == 0:
                                ps2 = pspool.tile([128, 2 * C], f32)
                            ps = ps2[:, (j % 2) * C : (j % 2 + 1) * C]
                        else:
                            ps = pspool.tile([128, C], f32, name="ps")[:]
                        for ch in range(CH):
                            m = mpool.tile([Kc, 128], fdt)
                            eq_eng = nc.gpsimd if EQ_ENG == "gpsimd" else nc.vector
                            eq_eng.tensor_scalar(
                                m[:],
                                iota_t[:Kc, :],
                                lidx_t[:, ch * NBLK + b : ch * NBLK + b + 1],
                                None,
                                op0=eq,
                            )
                            if split:
                                base = (gi * G + j) * 2 * C
                                nc.tensor.matmul(
                                    ps, m[:], ftiles[ch][:, base : base + C],
                                    start=(ch == 0), stop=False,
                                )
                                nc.tensor.matmul(
                                    ps, m[:], ftiles[ch][:, base + C : base + 2 * C],
                                    start=False, stop=(ch == CH - 1),
                                )
                            else:
                                lhsT = m[:]
                                rhs = ftiles[ch][
                                    :, (gi * G + j) * C : (gi * G + j + 1) * C
                                ]
                                if MM_DTYPE == "float32r":
                                    lhsT = lhsT.bitcast(mybir.dt.float32r)
                                    rhs = rhs.bitcast(mybir.dt.float32r)
                                nc.tensor.matmul(
                                    ps,
                                    lhsT,
                                    rhs,
                                    start=(ch == 0),
                                    stop=(ch == CH - 1),
                                )
                        # PSUM -> SBUF evacuation split between ACT and DVE so
                        # neither copy stream becomes the critical path
                        if CPAIR and G >= 2:
                            if j % 2 == 1:
                                pr = b // 2
                                dst = ot[:, (j - 1) * C : (j + 1) * C]
                                if (
                                    COPY_ENG == "mix" and pr % COPY_SCN == 0
                                ) or COPY_ENG == "act":
                                    nc.scalar.copy(dst, ps2[:])
                                else:
                                    nc.vector.tensor_copy(dst, ps2[:])
                        elif (COPY_ENG == "mix" and b % COPY_SCN == 0) or COPY_ENG == "act":
                            nc.scalar.copy(ot[:, j * C : (j + 1) * C], ps)
                        else:
                            nc.vector.tensor_copy(ot[:, j * C : (j + 1) * C], ps)
                        # half-group early store: Q10 starts draining sooner
                        if g < EARLY_SPLIT and G >= 2 and j == G // 2 - 1:
                            h = G // 2 * C
                            nc.scalar.dma_start(out_v[g][:, :h], ot[:, :h])
                    # store on the second HWDGE ring (ACT) to decouple from loads;
                    # optionally rebalance a fraction onto the sync ring
                    if g < EARLY_SPLIT and G >= 2:
                        h = G // 2 * C
                        nc.scalar.dma_start(out_v[g][:, h:], ot[:, h:])
                    elif STORE_MIX and g % STORE_MIX == STORE_MIX - 1:
                        nc.sync.dma_start(out_v[g], ot[:])
                    else:
                        nc.scalar.dma_start(out_v[g], ot[:])

    nc.compile()
    return nc


def _block_decomposition(idx, G):
    core = idx >> 13  # // 8192
    local = idx & 8191
    g = local // (128 * G)  # position group
    rem = local % (128 * G)
    p = rem // G  # partition (position G-tuple)
    j = rem % G  # lane within tuple
    blk = g * G + j  # block id within core, 0..63
    gblk = core * NBLK + blk  # global block id, 0..511
    counts = np.bincount(gblk, minlength=N_CORES * NBLK)
    K = int(counts.max())
    CH = (K + 127) // 128
    Kc = -(-K // CH)  # ceil
    # Multiple of 32 keeps the HWDGE descriptor fan-out balanced across all
    # 16 SDMA engines (measured: Kc=92 concentrates loads on 4 engines and
    # costs +80 us; Kc=96 spreads them).
    Kc = (Kc + 31) & ~31
    return gblk, p, CH, Kc


def _prepare_inputs(input_features, site_indices):
    feats = np.ascontiguousarray(np.asarray(input_features, dtype=np.float32))
    idx = np.asarray(site_indices).astype(np.int64)
    n = idx.shape[0]
    assert feats.shape == (n, C)

    # The block composition (hence the padded capacity Kc) depends on the
    # lane count G; pick the G that minimizes transferred bytes for this
    # input, preferring larger DMA runs on ties.
    if G_ENV is not None:
        G = int(G_ENV)
        gblk, lpos, CH, Kc = _block_decomposition(idx, G)
    else:
        best = None
        for cand in (4, 2, 8):
            gblk_c, lpos_c, CH_c, Kc_c = _block_decomposition(idx, cand)
            if best is None or CH_c * Kc_c < best[0] * best[1]:
                best = (CH_c, Kc_c, cand, gblk_c, lpos_c)
        CH, Kc, G, gblk, lpos = best

    order = np.argsort(gblk, kind="stable")
    counts = np.bincount(gblk, minlength=N_CORES * NBLK)

    starts = np.zeros(N_CORES * NBLK, dtype=np.int64)
    np.cumsum(counts[:-1], out=starts[1:])
    slot = np.arange(n, dtype=np.int64) - np.repeat(starts, counts)

    g_sorted = gblk[order]
    core_s = g_sorted // NBLK
    blk_s = g_sorted % NBLK
    ch_s = slot // Kc
    k_s = slot - ch_s * Kc

    if MM_DTYPE in ("bf16split", "fp16split"):
        if MM_DTYPE == "bf16split":
            import ml_dtypes

            hdt = ml_dtypes.bfloat16
        else:
            hdt = np.float16
        feats_pack = np.zeros((N_CORES, CH, Kc, NBLK, 2, C), dtype=hdt)
        fs = feats[order]
        hi = fs.astype(hdt)
        lo = (fs - hi.astype(np.float32)).astype(hdt)
        feats_pack[core_s, ch_s, k_s, blk_s, 0, :] = hi
        feats_pack[core_s, ch_s, k_s, blk_s, 1, :] = lo
    else:
        if MM_DTYPE == "fp16":
            pdt = np.float16
        elif MM_DTYPE == "bf16":
            import ml_dtypes

            pdt = ml_dtypes.bfloat16
        else:
            pdt = np.float32
        feats_pack = np.zeros((N_CORES, CH, Kc, NBLK, C), dtype=pdt)
        feats_pack[core_s, ch_s, k_s, blk_s, :] = feats[order].astype(pdt)
    lidx_pack = np.full((N_CORES, CH, Kc, NBLK), -1.0, dtype=np.float32)
    lidx_pack[core_s, ch_s, k_s, blk_s] = lpos[order].astype(np.float32)

    iota = np.tile(np.arange(128, dtype=np.float32), (128, 1))

    in_maps = []
    for c in range(N_CORES):
        m = {"iota": iota}
        for ch in range(CH):
            m[f"feats{ch}"] = feats_pack[c, ch].reshape(Kc, -1)
            m[f"lidx{ch}"] = lidx_pack[c, ch]
        in_maps.append(m)
    return in_maps, CH, Kc, G


def run(input_features, site_indices, trace: bool = False):
    if PBK and MM_DTYPE in ("fp16", "bf16"):
        feats = np.ascontiguousarray(np.asarray(input_features, dtype=np.float32))
        idx = np.asarray(site_indices).astype(np.int64)
        prep = _prepare_inputs_pbk(feats, idx, int(G_ENV) if G_ENV else 4)
        if prep is not None:
            in_maps, kg, kbe = prep
            G = int(G_ENV) if G_ENV else 4
            fbufs = FBUFS or 8
            obufs = OBUFS or 8
            key = ("pbk", kg, kbe, G, NLB, fbufs, obufs, CONST_RING,
                   MM_DTYPE, COPY_ENG, STORE_MIX, COPY_SCN)
            if key not in _PROGRAM_CACHE:
                _PROGRAM_CACHE[key] = _build_program_pbk(kg, kbe, G, fbufs, obufs)
            nc = _PROGRAM_CACHE[key]
            res = run_bass_kernel_spmd(nc, in_maps, list(range(N_CORES)), trace=trace)
            out = np.concatenate(
                [res.results[c]["out"] for c in range(N_CORES)], axis=0
            )
            return out.reshape(B, L, C), res
    in_maps, CH, Kc, G = _prepare_inputs(input_features, site_indices)
    if MM_DTYPE in ("fp16", "bf16"):
        # ft chunk = LOAD_GRPS*G KiB/partition (16-bit), ot = G*2 KiB
        fbufs = FBUFS or 6
        obufs = OBUFS or 8
    else:
        fbufs = FBUFS or _BUFS[G][0]
        obufs = OBUFS or _BUFS[G][1]
    key = (
        CH, Kc, G, fbufs, obufs, CONST_RING, MM_DTYPE, COPY_ENG, STORE_MIX,
        LOAD_GRPS, EQ_ENG, COPY_SCN, ALT_LOADS, EARLY_SPLIT, CPAIR,
    )
    if key not in _PROGRAM_CACHE:
        _PROGRAM_CACHE[key] = _build_program(CH, Kc, G, fbufs, obufs)
    nc = _PROGRAM_CACHE[key]
    res = run_bass_kernel_spmd(nc, in_maps, list(range(N_CORES)), trace=trace)
    out = np.concatenate([res.results[c]["out"] for c in range(N_CORES)], axis=0)
    return out.reshape(B, L, C), res


def kernel(input_features, site_indices, batch_size, length):
    assert int(batch_size) == B and int(length) == L
    out, _ = run(input_features, site_indices, trace=False)
    return out

